# revision 46
# baseline (speedup 1.0000x reference)
"""AttnBlock (GroupNorm + 4-head d=128 self-attention + residual).

Full input x: [8, 512, 2048] fp32. Data-parallel over batch: core b computes
batch b entirely on-chip (no collectives).

Per-core math (C=512, L=2048, G=4 groups, NH=4 heads, HD=128):
  h  = groupnorm(x)          bf16; x stays resident fp32 (residual reuse)
  q  = wq @ h + bq           bf16 [d, l] head-major (PE-transposed weights)
  k  = wk @ h + bk           bf16 [d, l]
  vT = h^T @ wv^T + bv       bf16 [l, d]
  sT[k,q] = k_tile^T q       fp32 PSUM, two k-tiles per 2-bank group
  e = exp(s * scale)         one ACT instr per [128,1024] group -> bf16
  den: DVE bf16 pairwise-add tree over the 8 e-groups -> [128,512],
       one ones-matmul broadcasts the cross-partition sum
  rden = reciprocal_approx_fast(den)   (custom DVE op, ~51 ULP)
  attn = pav * rden          bf16
  out = wo @ attn + bo + x   (DVE STT fuses bias + residual)

All matmul operands bf16 (fp32 PSUM accumulation). Steady state is paced by
the ACT exp stream (~1.15us per [128,1024] group); PE rides just under it,
DVE (den tree + drains) well under.

Scheduling:
 - DMA order: wk0/wq0 row-blocks, then all of x, then wv, wk/wq rest, wo.
 - Groupnorm stats + apply run per channel-tile as x lands; k[0] and
   q[0,qc0] projections accumulate ct-by-ct in the (still unused)
   sA/sB/av PSUM banks so attention starts ~1us after h completes.
 - v projection is emitted just-in-time inside unit (0,0); head h+1's
   k/q projection rides unit (h,0)'s hooks; proj drains go on ACT
   (which idles during the PE-bound qc=0 region).
 - den/rden/normalize of unit U are deferred into unit U+1 (PE never
   waits on the DVE tree); out-projection of qc rides qc+1's first unit.

PSUM budget (8 banks): sA 2 + sB 2 + av 2 + pp 2.
"""

import os
import numpy as np

import concourse.bass as bass
import concourse.tile as tile
from concourse import bacc, mybir
from concourse.bass_utils import run_bass_kernel_spmd
from concourse.masks import make_identity

F32 = mybir.dt.float32
BF16 = mybir.dt.bfloat16

B, C, L = 8, 512, 2048
G = 4            # groupnorm groups; group size 128 == one partition tile
NH, HD = 4, 128  # heads, head dim
CT = C // 128    # 4 channel tiles
LC = L // 512    # 4 l-chunks of 512
LT = L // 128    # 16 l-tiles of 128
NG = LT // 2     # 8 score groups of 2 k-tiles
EPS = 1e-6
SM_SCALE = float(HD) ** -0.5

AFT = mybir.ActivationFunctionType
ALU = mybir.AluOpType


def build_attn_block(nc):
    x_d = nc.dram_tensor("x", [C, L], F32, kind="ExternalInput").ap()
    gs_d = nc.dram_tensor("gn_scale", [C], F32, kind="ExternalInput").ap()
    gb_d = nc.dram_tensor("gn_bias", [C], F32, kind="ExternalInput").ap()
    w_d = {}
    b_d = {}
    for nm in ("q", "k", "v", "o"):
        w_d[nm] = nc.dram_tensor(f"w{nm}", [C, C], F32, kind="ExternalInput").ap()
        b_d[nm] = nc.dram_tensor(f"b{nm}", [C], F32, kind="ExternalInput").ap()
    out_d = nc.dram_tensor("out", [C, L], F32, kind="ExternalOutput").ap()

    with tile.TileContext(nc) as tc:
        with (
            tc.tile_pool(name="const", bufs=1) as const,
            tc.tile_pool(name="wstage", bufs=2) as wstage,
            tc.tile_pool(name="wt", bufs=1) as wt,
            tc.tile_pool(name="big", bufs=1) as big,
            tc.tile_pool(name="small", bufs=4) as small,
            tc.tile_pool(name="epool", bufs=4) as epool,
            tc.tile_pool(name="tpool", bufs=6) as tpool,
            tc.tile_pool(name="cpool", bufs=2) as cpool,
            tc.tile_pool(name="psum", bufs=1, space="PSUM") as psum,
        ):
            # ---- constants ----
            identity = const.tile([128, 128], F32)
            make_identity(nc, identity)
            ones = const.tile([128, 128], F32)
            nc.vector.memset(ones, 1.0)
            ones_bf = const.tile([128, 128], BF16)
            nc.vector.tensor_copy(ones_bf, ones)
            eps_t = const.tile([128, 1], F32)
            nc.vector.memset(eps_t, EPS)

            # ---- big persistent tiles ----
            x_sb = big.tile([128, CT, L], F32, tag="x_sb")
            h_sb = big.tile([128, CT, L], BF16, tag="h_sb")
            q_sb = big.tile([128, NH, L], BF16, tag="q_sb")
            k_sb = big.tile([128, NH, L], BF16, tag="k_sb")
            vT_sb = big.tile([128, LT, C], BF16, tag="vT_sb")
            attn_sb = big.tile([128, NH, L], BF16, tag="attn_sb")

            # ---- weights: DMA row-blocks, PE-transpose into wT[c_in, c_out]
            #      (bf16), drained as one [128, 4, 128] strided DVE copy.
            wts = {}
            for nm in ("q", "k", "v", "o"):
                wts[nm] = wt.tile([128, CT, C], BF16, name=f"w{nm}t")
            def emit_weight(nm, gate_src, obs=(0, 1, 2, 3)):
                # one DMA per row-block set, gated behind `gate_src` via a
                # marker write into the stage tile (WAW dep) so it cannot
                # steal HBM bandwidth from earlier transfers
                nb = len(obs)
                stg = wstage.tile([128, nb, C], F32, tag="stg", name=f"stg{nm}")
                if not isinstance(gate_src, list):
                    gate_src = [gate_src]
                for gi, gs in enumerate(gate_src):
                    nc.vector.tensor_copy(stg[:, 0, gi : gi + 1], gs)
                nc.sync.dma_start(
                    out=stg,
                    in_=w_d[nm][obs[0] * 128 : (obs[-1] + 1) * 128, :].rearrange(
                        "(o p) c -> p o c", p=128
                    ),
                )
                for i, ot in enumerate(obs):
                    pt = psum.tile([128, 512], F32, tag="pp", bufs=2, name="pt")
                    for ct in range(CT):
                        nc.tensor.transpose(
                            pt[:, ct * 128 : (ct + 1) * 128],
                            stg[:, i, ct * 128 : (ct + 1) * 128],
                            identity,
                        )
                    dstw = wts[nm][:, :, ot * 128 : (ot + 1) * 128]
                    nc.vector.tensor_copy(
                        dstw, pt.rearrange("p (c t) -> p c t", c=CT)
                    )
                return stg

            x_r = x_d.rearrange("(t p) l -> p t l", p=128)

            # x first: four concurrent per-ct transfers split the full HBM
            # bandwidth; weights are gated behind their completion markers
            for ct in range(CT):
                nc.sync.dma_start(out=x_sb[:, ct, :], in_=x_r[:, ct, :])

            def load_cvec(name, ap_1d):
                t = const.tile([128, CT], F32, name=name)
                nc.sync.dma_start(out=t, in_=ap_1d.rearrange("(t p) -> p t", p=128))
                return t

            bq_sb = load_cvec("bq_sb", b_d["q"])
            bk_sb = load_cvec("bk_sb", b_d["k"])
            bo_sb = load_cvec("bo_sb", b_d["o"])
            gs_sb = load_cvec("gs_sb", gs_d)
            gb_sb = load_cvec("gb_sb", gb_d)

            bv_bc = const.tile([128, C], F32)  # bv broadcast across partitions
            nc.sync.dma_start(
                out=bv_bc,
                in_=bass.AP(
                    tensor=b_d["v"].tensor,
                    offset=b_d["v"].offset,
                    ap=[[0, 128]] + list(b_d["v"].ap),
                ),
            )

            # ---- groupnorm stats + apply, per channel tile as x lands ----
            for ct in range(CT):
                stats = small.tile([128, 4, 6], F32, tag="stats")
                for i in range(4):
                    nc.vector.bn_stats(
                        out=stats[:, i, :], in_=x_sb[:, ct, i * 512 : (i + 1) * 512]
                    )
                mv = small.tile([128, 2], F32, tag="mv")
                nc.vector.bn_aggr(out=mv, in_=stats)
                stat2 = small.tile([128, 2], F32, tag="stat2")
                nc.vector.tensor_copy(stat2[:, 0:1], mv[:, 0:1])
                nc.vector.scalar_tensor_tensor(
                    out=stat2[:, 1:2],
                    in0=mv[:, 0:1],
                    scalar=mv[:, 0:1],
                    in1=mv[:, 1:2],
                    op0=ALU.mult,
                    op1=ALU.add,
                )
                pg = psum.tile([128, 2], F32, tag="av", bufs=2, name="pg")
                nc.tensor.matmul(pg, ones, stat2, start=True, stop=True)
                mean_t = small.tile([128, 1], F32, tag="mean_t")
                nc.vector.tensor_scalar_mul(mean_t, pg[:, 0:1], 1.0 / 128.0)
                ex2_t = small.tile([128, 1], F32, tag="ex2_t")
                nc.vector.tensor_scalar_mul(ex2_t, pg[:, 1:2], 1.0 / 128.0)
                var_t = small.tile([128, 1], F32, tag="var_t")
                nc.vector.tensor_mul(var_t, mean_t, mean_t)
                nc.vector.tensor_sub(var_t, ex2_t, var_t)
                std_t = small.tile([128, 1], F32, tag="std_t")
                nc.scalar.activation(std_t, var_t, AFT.Sqrt, bias=eps_t)
                rstd_t = small.tile([128, 1], F32, tag="rstd_t")
                nc.vector.reciprocal(rstd_t, std_t)
                a_t = small.tile([128, 1], F32, tag="a_t", bufs=CT)
                nc.vector.tensor_mul(a_t, rstd_t, gs_sb[:, ct : ct + 1])
                b_t = small.tile([128, 1], F32, tag="b_t", bufs=CT)
                nc.vector.tensor_mul(b_t, mean_t, a_t)
                nc.vector.tensor_sub(b_t, gb_sb[:, ct : ct + 1], b_t)
                # h for this channel tile (ACT, two instrs for latency)
                for i2 in range(2):
                    nc.scalar.activation(
                        h_sb[:, ct, i2 * 1024 : (i2 + 1) * 1024],
                        x_sb[:, ct, i2 * 1024 : (i2 + 1) * 1024],
                        AFT.Identity,
                        bias=b_t,
                        scale=a_t,
                    )
                if ct == CT - 1:
                    # preload the exp table set; the data-dep on std_t orders
                    # this AFTER every Sqrt so the set isn't evicted again
                    dummy = small.tile([128, 1], F32, tag="dummy")
                    nc.scalar.activation(dummy, std_t, AFT.Exp)

            # k/q weights (gated behind x), then head-0 projection into the
            # still-free sA/sB/av banks; v follows, o is emitted much later
            stg_k = emit_weight(
                "k", [x_sb[:, ct, L - 1 : L] for ct in range(CT)]
            )
            stg_q0 = emit_weight("q", stg_k[:, 0, 1:2], obs=(0,))
            ps_k0 = [
                psum.tile([128, 1024], F32, tag="sA", name="ps_k0a"),
                psum.tile([128, 1024], F32, tag="sB", name="ps_k0b"),
            ]
            for lc in range(LC):
                for ct in range(CT):
                    nc.tensor.matmul(
                        ps_k0[lc // 2][:, (lc % 2) * 512 : (lc % 2 + 1) * 512],
                        wts["k"][:, ct, 0:128],
                        h_sb[:, ct, lc * 512 : (lc + 1) * 512],
                        start=(ct == 0),
                        stop=(ct == CT - 1),
                    )
            ps_q0 = psum.tile([128, 512], F32, tag="av", bufs=2, name="ps_q0")
            for ct in range(CT):
                nc.tensor.matmul(
                    ps_q0,
                    wts["q"][:, ct, 0:128],
                    h_sb[:, ct, 0:512],
                    start=(ct == 0),
                    stop=(ct == CT - 1),
                )
            stg_v = emit_weight("v", stg_q0[:, 0, 1:2])
            stg_q = emit_weight("q", stg_v[:, 0, 1:2], obs=(1, 2, 3))

            # drain k[0] / q[0,lc0]: split ACT/DVE so neither serializes
            for lc in range(2):
                nc.scalar.activation(
                    k_sb[:, 0, lc * 512 : (lc + 1) * 512],
                    ps_k0[0][:, lc * 512 : (lc + 1) * 512],
                    AFT.Identity,
                    bias=bk_sb[:, 0:1],
                )
            for lc in range(2, LC):
                nc.vector.tensor_scalar_add(
                    k_sb[:, 0, lc * 512 : (lc + 1) * 512],
                    ps_k0[1][:, (lc - 2) * 512 : (lc - 1) * 512],
                    bk_sb[:, 0:1],
                )
            nc.vector.tensor_scalar_add(q_sb[:, 0, 0:512], ps_q0, bq_sb[:, 0:1])

            # ---- projection helpers (drains on DVE) ----
            def emit_proj_group(h, i):
                # i in 0..7: 0-3 -> k lc=i, 4-7 -> q lc=i-4
                dst, wtt, bias = (
                    (k_sb, wts["k"], bk_sb) if i < 4 else (q_sb, wts["q"], bq_sb)
                )
                lc = i % 4
                pp = psum.tile([128, 512], F32, tag="pp", bufs=2, name="pp")
                for ct in range(CT):
                    nc.tensor.matmul(
                        pp,
                        wtt[:, ct, h * 128 : (h + 1) * 128],
                        h_sb[:, ct, lc * 512 : (lc + 1) * 512],
                        start=(ct == 0),
                        stop=(ct == CT - 1),
                    )
                nc.vector.tensor_scalar_add(
                    dst[:, h, lc * 512 : (lc + 1) * 512], pp, bias[:, h : h + 1]
                )

            def emit_q0_group(lc):
                pp = psum.tile([128, 512], F32, tag="pp", bufs=2, name="pp")
                for ct in range(CT):
                    nc.tensor.matmul(
                        pp,
                        wts["q"][:, ct, 0:128],
                        h_sb[:, ct, lc * 512 : (lc + 1) * 512],
                        start=(ct == 0),
                        stop=(ct == CT - 1),
                    )
                nc.vector.tensor_scalar_add(
                    q_sb[:, 0, lc * 512 : (lc + 1) * 512], pp, bq_sb[:, 0:1]
                )

            def emit_v_tiles(lt0, n):
                for lt in range(lt0, lt0 + n):
                    pv = psum.tile([128, 512], F32, tag="pp", bufs=2, name="pv")
                    for ct in range(CT):
                        nc.tensor.matmul(
                            pv,
                            h_sb[:, ct, lt * 128 : (lt + 1) * 128],
                            wts["v"][:, ct, :],
                            start=(ct == 0),
                            stop=(ct == CT - 1),
                        )
                    nc.vector.tensor_add(vT_sb[:, lt, :], pv, bv_bc)

            def emit_out_proj_ot(qc, ot, pop=None):
                if pop is None:
                    pop = psum.tile([128, 512], F32, tag="pp", bufs=2, name="pop")
                for ct in range(CT):
                    nc.tensor.matmul(
                        pop,
                        wts["o"][:, ct, ot * 128 : (ot + 1) * 128],
                        attn_sb[:, ct, qc * 512 : (qc + 1) * 512],
                        start=(ct == 0),
                        stop=(ct == CT - 1),
                    )
                ot_sb = cpool.tile([128, 512], F32, tag="ot_sb")
                nc.vector.scalar_tensor_tensor(
                    out=ot_sb,
                    in0=pop,
                    scalar=bo_sb[:, ot : ot + 1],
                    in1=x_sb[:, ot, qc * 512 : (qc + 1) * 512],
                    op0=ALU.add,
                    op1=ALU.add,
                )
                nc.sync.dma_start(
                    out=out_d[ot * 128 : (ot + 1) * 128, qc * 512 : (qc + 1) * 512],
                    in_=ot_sb,
                )

            # ---- attention ----
            def emit_qk(h, qc, g):
                ps = psum.tile(
                    [128, 1024], F32, tag=("sA" if g % 2 == 0 else "sB"), name="ps"
                )
                for j in range(2):
                    kt = 2 * g + j
                    nc.tensor.matmul(
                        ps[:, j * 512 : (j + 1) * 512],
                        k_sb[:, h, kt * 128 : (kt + 1) * 128],
                        q_sb[:, h, qc * 512 : (qc + 1) * 512],
                        start=True,
                        stop=True,
                    )
                e = epool.tile([128, 1024], BF16, tag="e", bufs=4, name="e")
                nc.scalar.activation(e, ps, AFT.Exp, scale=SM_SCALE)
                return e

            def emit_av(h, pav, e, g):
                for j in range(2):
                    kt = 2 * g + j
                    nc.tensor.matmul(
                        pav,
                        vT_sb[:, kt, h * 128 : (h + 1) * 128],
                        e[:, j * 512 : (j + 1) * 512],
                        start=(kt == 0),
                        stop=(kt == LT - 1),
                    )

            def finish_unit(st):
                # den broadcast + 1/den + normalize for a completed unit;
                # deferred into the NEXT unit so the PE never waits on the
                # DVE tree.
                h, qc, pav, f = st
                pden = psum.tile([128, 512], F32, tag="pp", bufs=2, name="pden")
                nc.tensor.matmul(pden, ones_bf, f, start=True, stop=True)
                rden = cpool.tile([128, 512], F32, tag="rden", name="rden")
                nc.vector.reciprocal_approx_fast(rden, pden)
                nc.vector.tensor_mul(
                    attn_sb[:, h, qc * 512 : (qc + 1) * 512], pav, rden
                )

            def emit_unit(h, qc, hook=None, pe_den=False):
                es = []
                ts = []
                pav = psum.tile([128, 512], F32, tag="av", bufs=2, name="pav")
                pden = None
                if pe_den:
                    # last unit: accumulate den on the PE as e-groups land, so
                    # the finish isn't gated on the serial DVE tree at the tail
                    pden = psum.tile([128, 512], F32, tag="pp", bufs=2, name="pden")
                es.append(emit_qk(h, qc, 0))
                u0 = None
                for g in range(1, NG + 1):
                    if g < NG:
                        es.append(emit_qk(h, qc, g))
                    if hook is not None:
                        hook(g)
                    if pe_den:
                        # den before av: pden completes right after the last
                        # exp so the finish chain starts immediately
                        for j in range(2):
                            kt = 2 * (g - 1) + j
                            nc.tensor.matmul(
                                pden,
                                ones_bf,
                                es[g - 1][:, j * 512 : (j + 1) * 512],
                                start=(kt == 0),
                                stop=(kt == LT - 1),
                            )
                        emit_av(h, pav, es[g - 1], g - 1)
                        continue
                    emit_av(h, pav, es[g - 1], g - 1)
                    if g % 2 == 0:
                        t = tpool.tile([128, 1024], BF16, tag="t", bufs=6, name="t")
                        nc.vector.tensor_add(t, es[g - 2], es[g - 1])
                        ts.append(t)
                        if g == 4:
                            u0 = tpool.tile(
                                [128, 1024], BF16, tag="t", bufs=6, name="u0"
                            )
                            nc.vector.tensor_add(u0, ts[0], ts[1])
                if pe_den:
                    rden = cpool.tile([128, 512], F32, tag="rden", name="rden")
                    nc.vector.reciprocal_approx_fast(rden, pden)
                    nc.vector.tensor_mul(
                        attn_sb[:, h, qc * 512 : (qc + 1) * 512], pav, rden
                    )
                    return None
                u1 = tpool.tile([128, 1024], BF16, tag="t", bufs=6, name="u1")
                nc.vector.tensor_add(u1, ts[2], ts[3])
                s = tpool.tile([128, 1024], BF16, tag="t", bufs=6, name="s")
                nc.vector.tensor_add(s, u0, u1)
                f = tpool.tile([128, 512], BF16, tag="f", bufs=2, name="f")
                nc.vector.tensor_add(f, s[:, 0:512], s[:, 512:1024])
                return (h, qc, pav, f)

            pending = None  # completed unit awaiting den/normalize
            deferred_out = None  # qc whose out-projection awaits emission
            for qc in range(LC):
                for h in range(NH):
                    dq = deferred_out if h <= 1 and qc > 0 else None
                    if h == 1:
                        deferred_out = None
                    vjit = qc == 0 and h == 0
                    projh = h + 1 if (qc == 0 and 2 <= h + 1 <= 3) else None
                    pend = pending

                    def hook(g, _dq=dq, _vjit=vjit, _pend=pend, _projh=projh, _h=h):
                        if _vjit:
                            emit_v_tiles(2 * (g - 1), 2)
                            if g in (2, 4, 6):
                                emit_q0_group(g // 2)
                        elif _projh is not None:
                            emit_proj_group(_projh, g - 1)
                        if g == 2 and _pend is not None:
                            finish_unit(_pend)
                        if _dq is not None and g in (4, 6):
                            emit_out_proj_ot(_dq, _h * 2 + g // 2 - 2)

                    last = qc == LC - 1 and h == NH - 1
                    pending = emit_unit(h, qc, hook, pe_den=last)
                    if qc == 0 and h == 0:
                        # head 1's projection (unit 0,0's hooks carry v);
                        # wo arrives here so its transposes don't block the
                        # first unit's qk stream in the PE queue
                        for i in range(8):
                            emit_proj_group(1, i)
                        emit_weight("o", stg_q[:, 0, 1:2])
                deferred_out = qc
            # final out-projection: attention is over, so the score banks are
            # free — run all four ot-chains without pp contention
            ps_fin = [
                psum.tile([128, 1024], F32, tag="sA", name="fA"),
                psum.tile([128, 1024], F32, tag="sB", name="fB"),
            ]
            for ot in range(CT):
                emit_out_proj_ot(
                    deferred_out,
                    ot,
                    pop=ps_fin[ot // 2][:, (ot % 2) * 512 : (ot % 2 + 1) * 512],
                )
    nc.compile()
    return nc


_NC_CACHE = {}


def _get_nc():
    if "nc" not in _NC_CACHE:
        nc = bacc.Bacc("TRN2", debug=False)
        build_attn_block(nc)
        _NC_CACHE["nc"] = nc
    return _NC_CACHE["nc"]


def run(trace=False, **inputs):
    nc = _get_nc()
    xs = np.ascontiguousarray(np.asarray(inputs["x"], dtype=np.float32))
    shared = {}
    for nm in ("gn_scale", "gn_bias", "wq", "bq", "wk", "bk", "wv", "bv", "wo", "bo"):
        shared[nm] = np.ascontiguousarray(np.asarray(inputs[nm], dtype=np.float32))
    in_maps = [dict(shared, x=xs[b]) for b in range(B)]
    res = run_bass_kernel_spmd(nc, in_maps, core_ids=list(range(B)), trace=trace)
    out = np.stack([res.results[b]["out"] for b in range(B)], axis=0)
    return out, res


def kernel(**inputs):
    out, _ = run(trace=bool(os.environ.get("ATTN_TRACE")), **inputs)
    return out


# revision 48
# speedup vs baseline: 1.1817x; 1.1817x over previous
"""AttnBlock (GroupNorm + 4-head d=128 self-attention + residual).

Full input x: [8, 512, 2048] fp32. Data-parallel over batch: core b computes
batch b entirely on-chip (no collectives).

Per-core math (C=512, L=2048, G=4 groups, NH=4 heads, HD=128):
  h  = groupnorm(x)          bf16; x stays resident fp32 (residual reuse)
  q  = wq @ h + bq           bf16 [d, l] head-major (PE-transposed weights)
  k  = wk @ h + bk           bf16 [d, l]
  vT = h^T @ wv^T + bv       bf16 [l, d]
  sT[k,q] = k_tile^T q       fp32 PSUM, two k-tiles per 2-bank group
  e = exp(s * scale)         one ACT instr per [128,1024] group -> bf16
  den: DVE bf16 pairwise-add tree over the 8 e-groups -> [128,512],
       one ones-matmul broadcasts the cross-partition sum
  rden = reciprocal_approx_fast(den)   (custom DVE op, ~51 ULP)
  attn = pav * rden          bf16
  out = wo @ attn + bo + x   (DVE STT fuses bias + residual)

All matmul operands bf16 (fp32 PSUM accumulation). Steady state is paced by
the ACT exp stream (~1.15us per [128,1024] group); PE rides just under it,
DVE (den tree + drains) well under.

Scheduling:
 - DMA order: wk0/wq0 row-blocks, then all of x, then wv, wk/wq rest, wo.
 - Groupnorm stats + apply run per channel-tile as x lands; k[0] and
   q[0,qc0] projections accumulate ct-by-ct in the (still unused)
   sA/sB/av PSUM banks so attention starts ~1us after h completes.
 - v projection is emitted just-in-time inside unit (0,0); head h+1's
   k/q projection rides unit (h,0)'s hooks; proj drains go on ACT
   (which idles during the PE-bound qc=0 region).
 - den/rden/normalize of unit U are deferred into unit U+1 (PE never
   waits on the DVE tree); out-projection of qc rides qc+1's first unit.

PSUM budget (8 banks): sA 2 + sB 2 + av 2 + pp 2.
"""

import os
import numpy as np

import concourse.bass as bass
import concourse.tile as tile
from concourse import bacc, mybir
from concourse.bass_utils import run_bass_kernel_spmd
from concourse.masks import make_identity

F32 = mybir.dt.float32
BF16 = mybir.dt.bfloat16

B, C, L = 8, 512, 2048
G = 4            # groupnorm groups; group size 128 == one partition tile
NH, HD = 4, 128  # heads, head dim
CT = C // 128    # 4 channel tiles
LC = L // 512    # 4 l-chunks of 512
LT = L // 128    # 16 l-tiles of 128
NG = LT // 2     # 8 score groups of 2 k-tiles
EPS = 1e-6
SM_SCALE = float(HD) ** -0.5

AFT = mybir.ActivationFunctionType
ALU = mybir.AluOpType


def build_attn_block(nc):
    x_d = nc.dram_tensor("x", [C, L], F32, kind="ExternalInput").ap()
    gs_d = nc.dram_tensor("gn_scale", [C], F32, kind="ExternalInput").ap()
    gb_d = nc.dram_tensor("gn_bias", [C], F32, kind="ExternalInput").ap()
    w_d = {}
    b_d = {}
    for nm in ("q", "k", "v", "o"):
        w_d[nm] = nc.dram_tensor(f"w{nm}", [C, C], F32, kind="ExternalInput").ap()
        b_d[nm] = nc.dram_tensor(f"b{nm}", [C], F32, kind="ExternalInput").ap()
    out_d = nc.dram_tensor("out", [C, L], F32, kind="ExternalOutput").ap()

    with tile.TileContext(nc) as tc:
        with (
            tc.tile_pool(name="const", bufs=1) as const,
            tc.tile_pool(name="wstage", bufs=2) as wstage,
            tc.tile_pool(name="wt", bufs=1) as wt,
            tc.tile_pool(name="big", bufs=1) as big,
            tc.tile_pool(name="small", bufs=4) as small,
            tc.tile_pool(name="epool", bufs=4) as epool,
            tc.tile_pool(name="tpool", bufs=6) as tpool,
            tc.tile_pool(name="cpool", bufs=2) as cpool,
            tc.tile_pool(name="psum", bufs=1, space="PSUM") as psum,
        ):
            # ---- constants ----
            identity = const.tile([128, 128], F32)
            make_identity(nc, identity)
            ones = const.tile([128, 128], F32)
            nc.vector.memset(ones, 1.0)
            ones_bf = const.tile([128, 128], BF16)
            nc.vector.tensor_copy(ones_bf, ones)
            eps_t = const.tile([128, 1], F32)
            nc.vector.memset(eps_t, EPS)

            # ---- big persistent tiles ----
            x_sb = big.tile([128, CT, L], F32, tag="x_sb")
            h_sb = big.tile([128, CT, L], BF16, tag="h_sb")
            q_sb = big.tile([128, NH, L], BF16, tag="q_sb")
            k_sb = big.tile([128, NH, L], BF16, tag="k_sb")
            vT_sb = big.tile([128, LT, C], BF16, tag="vT_sb")
            attn_sb = big.tile([128, NH, L], BF16, tag="attn_sb")

            # ---- weights: DMA row-blocks, PE-transpose into wT[c_in, c_out]
            #      (bf16), drained as one [128, 4, 128] strided DVE copy.
            wts = {}
            for nm in ("q", "k", "v", "o"):
                wts[nm] = wt.tile([128, CT, C], BF16, name=f"w{nm}t")
            def emit_weight(nm, gate_src, obs=(0, 1, 2, 3)):
                # one DMA per row-block set, gated behind `gate_src` via a
                # marker write into the stage tile (WAW dep) so it cannot
                # steal HBM bandwidth from earlier transfers
                nb = len(obs)
                stg = wstage.tile([128, nb, C], F32, tag="stg", name=f"stg{nm}")
                if not isinstance(gate_src, list):
                    gate_src = [gate_src]
                for gi, gs in enumerate(gate_src):
                    nc.vector.tensor_copy(stg[:, 0, gi : gi + 1], gs)
                nc.sync.dma_start(
                    out=stg,
                    in_=w_d[nm][obs[0] * 128 : (obs[-1] + 1) * 128, :].rearrange(
                        "(o p) c -> p o c", p=128
                    ),
                )
                for i, ot in enumerate(obs):
                    pt = psum.tile([128, 512], F32, tag="pp", bufs=2, name="pt")
                    for ct in range(CT):
                        nc.tensor.transpose(
                            pt[:, ct * 128 : (ct + 1) * 128],
                            stg[:, i, ct * 128 : (ct + 1) * 128],
                            identity,
                        )
                    dstw = wts[nm][:, :, ot * 128 : (ot + 1) * 128]
                    nc.vector.tensor_copy(
                        dstw, pt.rearrange("p (c t) -> p c t", c=CT)
                    )
                return stg

            x_r = x_d.rearrange("(t p) l -> p t l", p=128)

            # x first, per-ct transfers chained sequentially (marker gates)
            # so channel tiles land in order and the groupnorm-stats
            # pipeline streams behind them
            for ct in range(CT):
                if ct > 0:
                    nc.vector.tensor_copy(
                        x_sb[:, ct, 0:1], x_sb[:, ct - 1, L - 1 : L]
                    )
                nc.sync.dma_start(out=x_sb[:, ct, :], in_=x_r[:, ct, :])

            def load_cvec(name, ap_1d):
                t = const.tile([128, CT], F32, name=name)
                nc.sync.dma_start(out=t, in_=ap_1d.rearrange("(t p) -> p t", p=128))
                return t

            bq_sb = load_cvec("bq_sb", b_d["q"])
            bk_sb = load_cvec("bk_sb", b_d["k"])
            bo_sb = load_cvec("bo_sb", b_d["o"])
            gs_sb = load_cvec("gs_sb", gs_d)
            gb_sb = load_cvec("gb_sb", gb_d)

            bv_bc = const.tile([128, C], F32)  # bv broadcast across partitions
            nc.sync.dma_start(
                out=bv_bc,
                in_=bass.AP(
                    tensor=b_d["v"].tensor,
                    offset=b_d["v"].offset,
                    ap=[[0, 128]] + list(b_d["v"].ap),
                ),
            )

            # ---- groupnorm stats + apply, per channel tile as x lands ----
            for ct in range(CT):
                stats = small.tile([128, 4, 6], F32, tag="stats")
                for i in range(4):
                    nc.vector.bn_stats(
                        out=stats[:, i, :], in_=x_sb[:, ct, i * 512 : (i + 1) * 512]
                    )
                mv = small.tile([128, 2], F32, tag="mv")
                nc.vector.bn_aggr(out=mv, in_=stats)
                stat2 = small.tile([128, 2], F32, tag="stat2")
                nc.vector.tensor_copy(stat2[:, 0:1], mv[:, 0:1])
                nc.vector.scalar_tensor_tensor(
                    out=stat2[:, 1:2],
                    in0=mv[:, 0:1],
                    scalar=mv[:, 0:1],
                    in1=mv[:, 1:2],
                    op0=ALU.mult,
                    op1=ALU.add,
                )
                pg = psum.tile([128, 2], F32, tag="av", bufs=2, name="pg")
                nc.tensor.matmul(pg, ones, stat2, start=True, stop=True)
                mean_t = small.tile([128, 1], F32, tag="mean_t")
                nc.vector.tensor_scalar_mul(mean_t, pg[:, 0:1], 1.0 / 128.0)
                ex2_t = small.tile([128, 1], F32, tag="ex2_t")
                nc.vector.tensor_scalar_mul(ex2_t, pg[:, 1:2], 1.0 / 128.0)
                var_t = small.tile([128, 1], F32, tag="var_t")
                nc.vector.tensor_mul(var_t, mean_t, mean_t)
                nc.vector.tensor_sub(var_t, ex2_t, var_t)
                std_t = small.tile([128, 1], F32, tag="std_t")
                nc.scalar.activation(std_t, var_t, AFT.Sqrt, bias=eps_t)
                rstd_t = small.tile([128, 1], F32, tag="rstd_t")
                nc.vector.reciprocal(rstd_t, std_t)
                a_t = small.tile([128, 1], F32, tag="a_t", bufs=CT)
                nc.vector.tensor_mul(a_t, rstd_t, gs_sb[:, ct : ct + 1])
                b_t = small.tile([128, 1], F32, tag="b_t", bufs=CT)
                nc.vector.tensor_mul(b_t, mean_t, a_t)
                nc.vector.tensor_sub(b_t, gb_sb[:, ct : ct + 1], b_t)
                # h for this channel tile (ACT, two instrs for latency)
                for i2 in range(2):
                    nc.scalar.activation(
                        h_sb[:, ct, i2 * 1024 : (i2 + 1) * 1024],
                        x_sb[:, ct, i2 * 1024 : (i2 + 1) * 1024],
                        AFT.Identity,
                        bias=b_t,
                        scale=a_t,
                    )
                if ct == CT - 1:
                    # preload the exp table set; the data-dep on std_t orders
                    # this AFTER every Sqrt so the set isn't evicted again
                    dummy = small.tile([128, 1], F32, tag="dummy")
                    nc.scalar.activation(dummy, std_t, AFT.Exp)

            # k/q weights (gated behind x), then head-0 projection into the
            # still-free sA/sB/av banks; v follows, o is emitted much later
            stg_k = emit_weight("k", x_sb[:, 2, L - 1 : L])
            stg_q0 = emit_weight("q", stg_k[:, 0, 1:2], obs=(0,))
            ps_k0 = [
                psum.tile([128, 1024], F32, tag="sA", name="ps_k0a"),
                psum.tile([128, 1024], F32, tag="sB", name="ps_k0b"),
            ]
            for lc in range(LC):
                for ct in range(CT):
                    nc.tensor.matmul(
                        ps_k0[lc // 2][:, (lc % 2) * 512 : (lc % 2 + 1) * 512],
                        wts["k"][:, ct, 0:128],
                        h_sb[:, ct, lc * 512 : (lc + 1) * 512],
                        start=(ct == 0),
                        stop=(ct == CT - 1),
                    )
            ps_q0 = psum.tile([128, 512], F32, tag="av", bufs=2, name="ps_q0")
            for ct in range(CT):
                nc.tensor.matmul(
                    ps_q0,
                    wts["q"][:, ct, 0:128],
                    h_sb[:, ct, 0:512],
                    start=(ct == 0),
                    stop=(ct == CT - 1),
                )
            stg_v = emit_weight("v", stg_q0[:, 0, 1:2])
            stg_q = emit_weight("q", stg_v[:, 0, 1:2], obs=(1, 2, 3))

            # drain k[0] / q[0,lc0]: split ACT/DVE so neither serializes
            for lc in range(2):
                nc.scalar.activation(
                    k_sb[:, 0, lc * 512 : (lc + 1) * 512],
                    ps_k0[0][:, lc * 512 : (lc + 1) * 512],
                    AFT.Identity,
                    bias=bk_sb[:, 0:1],
                )
            for lc in range(2, LC):
                nc.vector.tensor_scalar_add(
                    k_sb[:, 0, lc * 512 : (lc + 1) * 512],
                    ps_k0[1][:, (lc - 2) * 512 : (lc - 1) * 512],
                    bk_sb[:, 0:1],
                )
            nc.vector.tensor_scalar_add(q_sb[:, 0, 0:512], ps_q0, bq_sb[:, 0:1])

            # ---- projection helpers (drains on DVE) ----
            def emit_proj_group(h, i):
                # i in 0..7: 0-3 -> k lc=i, 4-7 -> q lc=i-4
                dst, wtt, bias = (
                    (k_sb, wts["k"], bk_sb) if i < 4 else (q_sb, wts["q"], bq_sb)
                )
                lc = i % 4
                pp = psum.tile([128, 512], F32, tag="pp", bufs=2, name="pp")
                for ct in range(CT):
                    nc.tensor.matmul(
                        pp,
                        wtt[:, ct, h * 128 : (h + 1) * 128],
                        h_sb[:, ct, lc * 512 : (lc + 1) * 512],
                        start=(ct == 0),
                        stop=(ct == CT - 1),
                    )
                nc.vector.tensor_scalar_add(
                    dst[:, h, lc * 512 : (lc + 1) * 512], pp, bias[:, h : h + 1]
                )

            def emit_q0_group(lc):
                pp = psum.tile([128, 512], F32, tag="pp", bufs=2, name="pp")
                for ct in range(CT):
                    nc.tensor.matmul(
                        pp,
                        wts["q"][:, ct, 0:128],
                        h_sb[:, ct, lc * 512 : (lc + 1) * 512],
                        start=(ct == 0),
                        stop=(ct == CT - 1),
                    )
                nc.vector.tensor_scalar_add(
                    q_sb[:, 0, lc * 512 : (lc + 1) * 512], pp, bq_sb[:, 0:1]
                )

            def emit_v_tiles(lt0, n):
                for lt in range(lt0, lt0 + n):
                    pv = psum.tile([128, 512], F32, tag="pp", bufs=2, name="pv")
                    for ct in range(CT):
                        nc.tensor.matmul(
                            pv,
                            h_sb[:, ct, lt * 128 : (lt + 1) * 128],
                            wts["v"][:, ct, :],
                            start=(ct == 0),
                            stop=(ct == CT - 1),
                        )
                    nc.vector.tensor_add(vT_sb[:, lt, :], pv, bv_bc)

            def emit_out_proj_ot(qc, ot, pop=None):
                if pop is None:
                    pop = psum.tile([128, 512], F32, tag="pp", bufs=2, name="pop")
                for ct in range(CT):
                    nc.tensor.matmul(
                        pop,
                        wts["o"][:, ct, ot * 128 : (ot + 1) * 128],
                        attn_sb[:, ct, qc * 512 : (qc + 1) * 512],
                        start=(ct == 0),
                        stop=(ct == CT - 1),
                    )
                ot_sb = cpool.tile([128, 512], F32, tag="ot_sb")
                nc.vector.scalar_tensor_tensor(
                    out=ot_sb,
                    in0=pop,
                    scalar=bo_sb[:, ot : ot + 1],
                    in1=x_sb[:, ot, qc * 512 : (qc + 1) * 512],
                    op0=ALU.add,
                    op1=ALU.add,
                )
                nc.sync.dma_start(
                    out=out_d[ot * 128 : (ot + 1) * 128, qc * 512 : (qc + 1) * 512],
                    in_=ot_sb,
                )

            # ---- attention ----
            def emit_qk(h, qc, g):
                ps = psum.tile(
                    [128, 1024], F32, tag=("sA" if g % 2 == 0 else "sB"), name="ps"
                )
                for j in range(2):
                    kt = 2 * g + j
                    nc.tensor.matmul(
                        ps[:, j * 512 : (j + 1) * 512],
                        k_sb[:, h, kt * 128 : (kt + 1) * 128],
                        q_sb[:, h, qc * 512 : (qc + 1) * 512],
                        start=True,
                        stop=True,
                    )
                e = epool.tile([128, 1024], BF16, tag="e", bufs=4, name="e")
                nc.scalar.activation(e, ps, AFT.Exp, scale=SM_SCALE)
                return e

            def emit_av(h, pav, e, g):
                for j in range(2):
                    kt = 2 * g + j
                    nc.tensor.matmul(
                        pav,
                        vT_sb[:, kt, h * 128 : (h + 1) * 128],
                        e[:, j * 512 : (j + 1) * 512],
                        start=(kt == 0),
                        stop=(kt == LT - 1),
                    )

            def finish_unit(st):
                # den broadcast + 1/den + normalize for a completed unit;
                # deferred into the NEXT unit so the PE never waits on the
                # DVE tree.
                h, qc, pav, f = st
                pden = psum.tile([128, 512], F32, tag="pp", bufs=2, name="pden")
                nc.tensor.matmul(pden, ones_bf, f, start=True, stop=True)
                rden = cpool.tile([128, 512], F32, tag="rden", name="rden")
                nc.vector.reciprocal_approx_fast(rden, pden)
                nc.vector.tensor_mul(
                    attn_sb[:, h, qc * 512 : (qc + 1) * 512], pav, rden
                )

            def emit_unit(h, qc, hook=None, pe_den=False):
                es = []
                ts = []
                pav = psum.tile([128, 512], F32, tag="av", bufs=2, name="pav")
                pden = None
                if pe_den:
                    # last unit: accumulate den on the PE as e-groups land, so
                    # the finish isn't gated on the serial DVE tree at the tail
                    pden = psum.tile([128, 512], F32, tag="pp", bufs=2, name="pden")
                es.append(emit_qk(h, qc, 0))
                u0 = None
                for g in range(1, NG + 1):
                    if g < NG:
                        es.append(emit_qk(h, qc, g))
                    if hook is not None:
                        hook(g)
                    if pe_den:
                        # den before av: pden completes right after the last
                        # exp so the finish chain starts immediately
                        for j in range(2):
                            kt = 2 * (g - 1) + j
                            nc.tensor.matmul(
                                pden,
                                ones_bf,
                                es[g - 1][:, j * 512 : (j + 1) * 512],
                                start=(kt == 0),
                                stop=(kt == LT - 1),
                            )
                        emit_av(h, pav, es[g - 1], g - 1)
                        continue
                    emit_av(h, pav, es[g - 1], g - 1)
                    if g % 2 == 0:
                        t = tpool.tile([128, 1024], BF16, tag="t", bufs=6, name="t")
                        nc.vector.tensor_add(t, es[g - 2], es[g - 1])
                        ts.append(t)
                        if g == 4:
                            u0 = tpool.tile(
                                [128, 1024], BF16, tag="t", bufs=6, name="u0"
                            )
                            nc.vector.tensor_add(u0, ts[0], ts[1])
                if pe_den:
                    rden = cpool.tile([128, 512], F32, tag="rden", name="rden")
                    nc.vector.reciprocal_approx_fast(rden, pden)
                    nc.vector.tensor_mul(
                        attn_sb[:, h, qc * 512 : (qc + 1) * 512], pav, rden
                    )
                    return None
                u1 = tpool.tile([128, 1024], BF16, tag="t", bufs=6, name="u1")
                nc.vector.tensor_add(u1, ts[2], ts[3])
                s = tpool.tile([128, 1024], BF16, tag="t", bufs=6, name="s")
                nc.vector.tensor_add(s, u0, u1)
                f = tpool.tile([128, 512], BF16, tag="f", bufs=2, name="f")
                nc.vector.tensor_add(f, s[:, 0:512], s[:, 512:1024])
                return (h, qc, pav, f)

            pending = None  # completed unit awaiting den/normalize
            deferred_out = None  # qc whose out-projection awaits emission
            for qc in range(LC):
                for h in range(NH):
                    dq = deferred_out if h <= 1 and qc > 0 else None
                    if h == 1:
                        deferred_out = None
                    vjit = qc == 0 and h == 0
                    projh = h + 1 if (qc == 0 and 2 <= h + 1 <= 3) else None
                    pend = pending

                    def hook(g, _dq=dq, _vjit=vjit, _pend=pend, _projh=projh, _h=h):
                        if _vjit:
                            emit_v_tiles(2 * (g - 1), 2)
                            if g in (2, 4, 6):
                                emit_q0_group(g // 2)
                        elif _projh is not None:
                            emit_proj_group(_projh, g - 1)
                        if g == 2 and _pend is not None:
                            finish_unit(_pend)
                        if _dq is not None and g in (4, 6):
                            emit_out_proj_ot(_dq, _h * 2 + g // 2 - 2)

                    last = qc == LC - 1 and h == NH - 1
                    pending = emit_unit(h, qc, hook, pe_den=last)
                    if qc == 0 and h == 0:
                        # head 1's projection (unit 0,0's hooks carry v);
                        # wo arrives here so its transposes don't block the
                        # first unit's qk stream in the PE queue
                        for i in range(8):
                            emit_proj_group(1, i)
                        emit_weight("o", stg_q[:, 0, 1:2])
                deferred_out = qc
            # final out-projection: attention is over, so the score banks are
            # free — run all four ot-chains without pp contention
            ps_fin = [
                psum.tile([128, 1024], F32, tag="sA", name="fA"),
                psum.tile([128, 1024], F32, tag="sB", name="fB"),
            ]
            for ot in range(CT):
                emit_out_proj_ot(
                    deferred_out,
                    ot,
                    pop=ps_fin[ot // 2][:, (ot % 2) * 512 : (ot % 2 + 1) * 512],
                )
    nc.compile()
    return nc


_NC_CACHE = {}


def _get_nc():
    if "nc" not in _NC_CACHE:
        nc = bacc.Bacc("TRN2", debug=False)
        build_attn_block(nc)
        _NC_CACHE["nc"] = nc
    return _NC_CACHE["nc"]


def run(trace=False, **inputs):
    nc = _get_nc()
    xs = np.ascontiguousarray(np.asarray(inputs["x"], dtype=np.float32))
    shared = {}
    for nm in ("gn_scale", "gn_bias", "wq", "bq", "wk", "bk", "wv", "bv", "wo", "bo"):
        shared[nm] = np.ascontiguousarray(np.asarray(inputs[nm], dtype=np.float32))
    in_maps = [dict(shared, x=xs[b]) for b in range(B)]
    res = run_bass_kernel_spmd(nc, in_maps, core_ids=list(range(B)), trace=trace)
    out = np.stack([res.results[b]["out"] for b in range(B)], axis=0)
    return out, res


def kernel(**inputs):
    out, _ = run(trace=bool(os.environ.get("ATTN_TRACE")), **inputs)
    return out
